# revision 3
# baseline (speedup 1.0000x reference)
"""Trainium2 Bass kernel for nn_CuPyLinear: y = x @ W^T with W given as a
sparse COO weight (data, rows, cols), W [4096, 4096] ~10% dense,
x [2, 2048, 4096] fp32, y [2, 2048, 4096] fp32.

Strategy (8 NeuronCores, SPMD single NEFF):
  core c -> output-feature quarter q = c % 4 (1024 rows of W) and token half
  h = c // 4 (2048 tokens).  Host-side work is layout only: shard/sort/pad
  the COO indices, transpose/cast x; every arithmetic combination of data
  values (duplicate-COO summation, densification, matmul) runs on device.

Per-core device program:
  - Scatter arrays in NGRP=4 groups of GB=8 i-blocks (an i-block is 128
    input features):
      sk/sv [NGRP, 128, GHOST+GB*L] fp32: sort key / COO value lists, where
        the entry for W[r, c] sits in partition p = c % 128, i-block
        b = c // 128, sorted by r; duplicate (r, c) entries are adjacent.
        Pads/ghost use key = -1, val = 0.
      si [NGRP, 128, GB*L] int16: scatter index (r offset within the
        quarter); for duplicate runs all but the LAST entry are -1 (killed).
  - Duplicate merge per group on VectorE: mask = (key == shift1(key)), then
    tensor_tensor_scan computes state = mask*state + val along the free dim
    (fp32 state), i.e. segmented running sums; the last entry of each
    duplicate run holds the full sum.  Output downcast to bf16.
  - GpSimd local_scatter builds dense W^T tiles wt[b] [128 i x 1024 o] bf16
    (killed/pad indices are skipped, so each surviving entry carries its
    run's sum).
  - TensorE: yT[o, t] += wt[b][:, ob*128:+128].T @ xT[b][:, ts*512:+512]
    accumulating GB i-blocks per group into 4-bank PSUM chunks [128, 2048]
    fp32; chunks are downcast bf16 and combined across groups in DRAM with
    GpSimd CCE accumulate-DMA.
  - Host assembles y from the per-core yT slices (cast fp32, transpose).
"""

import numpy as np
import ml_dtypes

import concourse.bacc as bacc
import concourse.mybir as mybir
import concourse.tile as tile
from concourse import library_config
from concourse.bass_utils import run_bass_kernel_spmd

P = 128
IN_F = 4096
OUT_F = 4096
BATCH, SEQ = 2, 2048
NB = 32            # i-blocks of 128 input features
GHOST = 16
OSH = 1024         # out features per core (quarter)
TSH = 2048         # tokens per core (half)
NGRP = 4
GB = NB // NGRP    # i-blocks per group
NOB = OSH // P     # output blocks per core
NTS = TSH // 512   # token slices per core


def _build_nc(L, reps=1):
    GW = GB * L
    SW = GHOST + GW
    nc = bacc.Bacc("TRN2", target_bir_lowering=False, debug=False, num_devices=8)
    dt = mybir.dt
    xs_d = nc.dram_tensor("xs", [NGRP, P, GB, TSH], dt.bfloat16,
                          kind="ExternalInput").ap()
    sk_d = nc.dram_tensor("sk", [NGRP, P, SW], dt.float32,
                          kind="ExternalInput").ap()
    sv_d = nc.dram_tensor("sv", [NGRP, P, SW], dt.float32,
                          kind="ExternalInput").ap()
    si_d = nc.dram_tensor("si", [NGRP, P, GW], dt.int16,
                          kind="ExternalInput").ap()
    y_d = nc.dram_tensor("y", [OSH, TSH], dt.bfloat16,
                         kind="ExternalOutput").ap()

    with tile.TileContext(nc) as tc:
        with tc.tile_pool(name="mrg", bufs=2) as mrgp, \
             tc.tile_pool(name="scat", bufs=2) as scatp, \
             tc.tile_pool(name="wt", bufs=1) as wtp, \
             tc.tile_pool(name="xstr", bufs=2) as xsp, \
             tc.tile_pool(name="st", bufs=3) as stp, \
             tc.tile_pool(name="psum", bufs=2, space="PSUM") as psp:

            nc.gpsimd.load_library(library_config.local_scatter)

            wt = [wtp.tile([P, OSH], dt.bfloat16, name=f"wt{b}")
                  for b in range(NB)]

            for _ in range(reps):
                for g in range(NGRP):
                    skt = scatp.tile([P, SW], dt.float32, name="skt")
                    svt = scatp.tile([P, SW], dt.float32, name="svt")
                    sit = scatp.tile([P, GW], dt.int16, name="sit")
                    nc.sync.dma_start(skt[:], sk_d[g])
                    nc.sync.dma_start(svt[:], sv_d[g])
                    nc.sync.dma_start(sit[:], si_d[g])
                    tm = mrgp.tile([P, GW], dt.float32, name="mask")
                    nc.vector.tensor_tensor(
                        out=tm[:], in0=skt[:, GHOST:], in1=skt[:, GHOST - 1:SW - 1],
                        op=mybir.AluOpType.is_equal)
                    vb = mrgp.tile([P, GW], dt.bfloat16, name="vb")
                    nc.vector.tensor_tensor_scan(
                        out=vb[:], data0=tm[:], data1=svt[:, GHOST:],
                        initial=0.0, op0=mybir.AluOpType.mult,
                        op1=mybir.AluOpType.add)
                    for j in range(GB):
                        b = g * GB + j
                        nc.gpsimd.local_scatter(
                            wt[b][:], vb[:, j * L:(j + 1) * L],
                            sit[:, j * L:(j + 1) * L],
                            channels=P, num_elems=OSH, num_idxs=L)

                for g in range(NGRP):
                    xst = xsp.tile([P, GB, TSH], dt.bfloat16, name="xst")
                    nc.sync.dma_start(xst[:], xs_d[g])
                    for ob in range(NOB):
                        pt = psp.tile([P, NTS * 512], dt.float32, name="ps")
                        for ts in range(NTS):
                            for j in range(GB):
                                b = g * GB + j
                                nc.tensor.matmul(
                                    pt[:, ts * 512:(ts + 1) * 512],
                                    lhsT=wt[b][:, ob * P:(ob + 1) * P],
                                    rhs=xst[:, j, ts * 512:(ts + 1) * 512],
                                    start=(j == 0), stop=(j == GB - 1))
                        st = stp.tile([P, NTS * 512], dt.bfloat16, name=f"st{ob % 3}")
                        nc.vector.tensor_copy(st[:], pt[:])
                        if g == 0:
                            nc.sync.dma_start(y_d[ob * P:(ob + 1) * P, :], st[:])
                        else:
                            nc.gpsimd.dma_start(y_d[ob * P:(ob + 1) * P, :], st[:],
                                                accum_op=mybir.AluOpType.add)

    nc.compile()
    return nc


def _make_scatter_arrays(rows, cols, data, o_base, L):
    m = (rows >= o_base) & (rows < o_base + OSH)
    r = (rows[m] - o_base).astype(np.int64)
    c = cols[m].astype(np.int64)
    v = data[m].astype(np.float32)
    p = c % P
    b = c // P
    group = p * NB + b
    order = np.lexsort((r, group))
    r, v, p, b, group = r[order], v[order], p[order], b[order], group[order]
    counts = np.bincount(group, minlength=P * NB)
    assert counts.max() <= L - 1, (counts.max(), L)
    starts = np.zeros(P * NB, np.int64)
    starts[1:] = np.cumsum(counts)[:-1]
    pos = np.arange(len(r)) - starts[group]

    key = np.full((P, NB, L), -1.0, np.float32)
    val = np.zeros((P, NB, L), np.float32)
    idxk = np.full((P, NB, L), -1, np.int16)
    # keys globally distinct across i-blocks -> no false merges across block
    # boundaries within a partition row
    key[p, b, pos] = (b * OSH + r).astype(np.float32)
    val[p, b, pos] = v
    nonlast = np.zeros(len(r), bool)
    nonlast[:-1] = (group[:-1] == group[1:]) & (r[:-1] == r[1:])
    ent = r.astype(np.int16)
    ent[nonlast] = -1
    idxk[p, b, pos] = ent

    GW = GB * L
    sk = np.full((NGRP, P, GHOST + GW), -1.0, np.float32)
    sv = np.zeros((NGRP, P, GHOST + GW), np.float32)
    si = np.ascontiguousarray(idxk.reshape(P, NGRP, GW).transpose(1, 0, 2))
    sk[:, :, GHOST:] = key.reshape(P, NGRP, GW).transpose(1, 0, 2)
    sv[:, :, GHOST:] = val.reshape(P, NGRP, GW).transpose(1, 0, 2)
    return sk, sv, si


def _choose_L(rows, cols):
    worst = 0
    for q in range(4):
        m = (rows >= q * OSH) & (rows < (q + 1) * OSH)
        c = cols[m].astype(np.int64)
        group = (c % P) * np.int64(NB) + (c // P)
        counts = np.bincount(group, minlength=P * NB)
        worst = max(worst, int(counts.max()))
    L = worst + 2
    L += L % 2
    return L


def _prep_x_half(xf, h):
    xh = xf[h * TSH:(h + 1) * TSH]                      # [t, k]
    a = xh.T.reshape(NGRP, GB, P, TSH)                  # [g, j, p, t]
    a = np.ascontiguousarray(a.transpose(0, 2, 1, 3))   # [g, p, j, t]
    return a.astype(ml_dtypes.bfloat16)


def _make_in_maps(x, data, rows, cols, L):
    x = np.asarray(x, dtype=np.float32)
    data = np.asarray(data, dtype=np.float32)
    rows = np.asarray(rows).astype(np.int64)
    cols = np.asarray(cols).astype(np.int64)

    xf = np.ascontiguousarray(x.reshape(-1, IN_F))
    xs_halves = [_prep_x_half(xf, h) for h in range(2)]
    scat = [_make_scatter_arrays(rows, cols, data, q * OSH, L)
            for q in range(4)]

    in_maps = []
    for core in range(8):
        q, h = core % 4, core // 4
        sk, sv, si = scat[q]
        in_maps.append({"xs": xs_halves[h], "sk": sk, "sv": sv, "si": si})
    return in_maps


def kernel(x, data, rows, cols):
    L = _choose_L(rows, cols)
    in_maps = _make_in_maps(x, data, rows, cols, L)
    nc = _build_nc(L)
    res = run_bass_kernel_spmd(nc, in_maps, core_ids=list(range(8)))

    y = np.zeros((BATCH * SEQ, OUT_F), np.float32)
    for core in range(8):
        q, h = core % 4, core // 4
        y[h * TSH:(h + 1) * TSH, q * OSH:(q + 1) * OSH] = \
            res.results[core]["y"].astype(np.float32).T
    return y.reshape(BATCH, SEQ, OUT_F)



# revision 4
# speedup vs baseline: 335.4769x; 335.4769x over previous
"""Trainium2 Bass kernel for nn_CuPyLinear: y = x @ W^T with W given as a
sparse COO weight (data, rows, cols), W [4096, 4096] ~10% dense,
x [2, 2048, 4096] fp32, y [2, 2048, 4096] fp32.

Strategy (8 NeuronCores, SPMD single NEFF): core c -> output-feature octant
(512 rows of W); every core sees all 4096 tokens.  Host-side work is layout
only: shard/sort/pad the COO indices, transpose/cast x; every arithmetic
combination of data values (duplicate-COO summation, densification, matmul)
runs on device.

Per-core device program:
  - Scatter arrays in NGRP=4 groups of GB=8 i-blocks (an i-block is 128
    input features):
      sk/sv [NGRP, 128, GHOST+GB*L] fp32: sort key / COO value lists, where
        the entry for W[r, c] sits in partition p = c % 128, i-block
        b = c // 128, sorted by r; duplicate (r, c) entries are adjacent.
        Pads/ghost use key = -1, val = 0.
      si [NGRP, 128, GB*L] int16: scatter index (r offset within the
        octant); for duplicate runs all but the LAST entry are -1 (killed).
  - Duplicate merge per group on VectorE: mask = (key == shift1(key)), then
    tensor_tensor_scan computes segmented running sums; the last entry of
    each duplicate run holds the full sum.  Output downcast to bf16.
  - GpSimd local_scatter builds dense W^T tiles wt[b] [128 i x 512 o] bf16.
  - TensorE: yT[o, t] += wt[b][:, ob*128:+128].T @ x[b-slice, ts*512:+512]
    with full-K accumulation in PSUM (32 i-blocks per accumulation group).
    Token slices ts0/ts1 are computed DURING the scatter phase (their
    matmuls chase the scatter group by group); ts2..7 run b-inner after.
  - y written once per (ob, ts) tile: PSUM -> bf16 -> DMA.  No DRAM
    accumulation.
  - Host assembles y from the per-core yT slices (cast fp32, transpose).
"""

import numpy as np
import ml_dtypes

import concourse.bacc as bacc
import concourse.mybir as mybir
import concourse.tile as tile
from concourse import library_config
from concourse.bass_utils import run_bass_kernel_spmd

P = 128
IN_F = 4096
OUT_F = 4096
BATCH, SEQ = 2, 2048
NTOK = BATCH * SEQ
NB = 32            # i-blocks of 128 input features
GHOST = 16
OSH = 512          # out features per core (octant)
NGRP = 4
GB = NB // NGRP    # i-blocks per merge group
NOB = OSH // P     # output 128-blocks per core (4)
TS = 512           # token slice width
NTS = NTOK // TS   # token slices (8)
NTS_A = 2          # token slices computed during the scatter phase


def _build_nc(L, reps=1):
    GW = GB * L
    SW = GHOST + GW
    nc = bacc.Bacc("TRN2", target_bir_lowering=False, debug=False, num_devices=8)
    dt = mybir.dt
    xs_d = nc.dram_tensor("xs", [NTS, P, NB, TS], dt.bfloat16,
                          kind="ExternalInput").ap()
    sk_d = nc.dram_tensor("sk", [NGRP, P, SW], dt.float32,
                          kind="ExternalInput").ap()
    sv_d = nc.dram_tensor("sv", [NGRP, P, SW], dt.float32,
                          kind="ExternalInput").ap()
    si_d = nc.dram_tensor("si", [NGRP, P, GW], dt.int16,
                          kind="ExternalInput").ap()
    y_d = nc.dram_tensor("y", [OSH, NTOK], dt.bfloat16,
                         kind="ExternalOutput").ap()

    with tile.TileContext(nc) as tc:
        with tc.tile_pool(name="mrg", bufs=2) as mrgp, \
             tc.tile_pool(name="scat", bufs=2) as scatp, \
             tc.tile_pool(name="wt", bufs=1) as wtp, \
             tc.tile_pool(name="xstr", bufs=3) as xsp, \
             tc.tile_pool(name="st", bufs=3) as stp, \
             tc.tile_pool(name="psum", bufs=2, space="PSUM") as psp:

            nc.gpsimd.load_library(library_config.local_scatter)

            wt = [wtp.tile([P, OSH], dt.bfloat16, name=f"wt{b}")
                  for b in range(NB)]

            for _ in range(reps):
                # x for the overlap slices, on the scalar-engine DMA queue so
                # the sync queue serves the scatter arrays first.
                xst = {}
                for ts in range(NTS_A):
                    xst[ts] = xsp.tile([P, NB, TS], dt.bfloat16, name="xst")
                    nc.scalar.dma_start(xst[ts][:], xs_d[ts])

                pacc = [psp.tile([P, NOB, TS], dt.float32, name="pacc")
                        for _ in range(NTS_A)]

                # Phase 1 + A: scatter group g, then immediately chase it
                # with the (ts0, ts1) matmuls over that group's i-blocks.
                for g in range(NGRP):
                    skt = scatp.tile([P, SW], dt.float32, name="skt")
                    svt = scatp.tile([P, SW], dt.float32, name="svt")
                    sit = scatp.tile([P, GW], dt.int16, name="sit")
                    nc.sync.dma_start(skt[:], sk_d[g])
                    nc.sync.dma_start(svt[:], sv_d[g])
                    nc.sync.dma_start(sit[:], si_d[g])
                    tm = mrgp.tile([P, GW], dt.float32, name="mask")
                    nc.vector.tensor_tensor(
                        out=tm[:], in0=skt[:, GHOST:], in1=skt[:, GHOST - 1:SW - 1],
                        op=mybir.AluOpType.is_equal)
                    vb = mrgp.tile([P, GW], dt.bfloat16, name="vb")
                    nc.vector.tensor_tensor_scan(
                        out=vb[:], data0=tm[:], data1=svt[:, GHOST:],
                        initial=0.0, op0=mybir.AluOpType.mult,
                        op1=mybir.AluOpType.add)
                    for j in range(GB):
                        b = g * GB + j
                        nc.gpsimd.local_scatter(
                            wt[b][:], vb[:, j * L:(j + 1) * L],
                            sit[:, j * L:(j + 1) * L],
                            channels=P, num_elems=OSH, num_idxs=L)
                    for ts in range(NTS_A):
                        for ob in range(NOB):
                            for j in range(GB):
                                b = g * GB + j
                                nc.tensor.matmul(
                                    pacc[ts][:, ob, :],
                                    lhsT=wt[b][:, ob * P:(ob + 1) * P],
                                    rhs=xst[ts][:, b, :],
                                    start=(g == 0 and j == 0),
                                    stop=(g == NGRP - 1 and j == GB - 1))

                for ts in range(NTS_A):
                    st = stp.tile([P, NOB, TS], dt.bfloat16, name="st")
                    nc.vector.tensor_copy(st[:], pacc[ts][:])
                    for ob in range(NOB):
                        nc.sync.dma_start(
                            y_d[ob * P:(ob + 1) * P, ts * TS:(ts + 1) * TS],
                            st[:, ob, :])

                # Phase B: remaining token slices, full-K b-inner.
                for ts in range(NTS_A, NTS):
                    xt = xsp.tile([P, NB, TS], dt.bfloat16, name="xst")
                    nc.scalar.dma_start(xt[:], xs_d[ts])
                    pt = psp.tile([P, NOB, TS], dt.float32, name="pacc")
                    for ob in range(NOB):
                        for b in range(NB):
                            nc.tensor.matmul(
                                pt[:, ob, :],
                                lhsT=wt[b][:, ob * P:(ob + 1) * P],
                                rhs=xt[:, b, :],
                                start=(b == 0), stop=(b == NB - 1))
                    st = stp.tile([P, NOB, TS], dt.bfloat16, name="st")
                    nc.vector.tensor_copy(st[:], pt[:])
                    for ob in range(NOB):
                        nc.sync.dma_start(
                            y_d[ob * P:(ob + 1) * P, ts * TS:(ts + 1) * TS],
                            st[:, ob, :])

    nc.compile()
    return nc


def _make_scatter_arrays(rows, cols, data, o_base, L):
    m = (rows >= o_base) & (rows < o_base + OSH)
    r = (rows[m] - o_base).astype(np.int64)
    c = cols[m].astype(np.int64)
    v = data[m].astype(np.float32)
    p = c % P
    b = c // P
    group = p * NB + b
    order = np.lexsort((r, group))
    r, v, p, b, group = r[order], v[order], p[order], b[order], group[order]
    counts = np.bincount(group, minlength=P * NB)
    assert counts.max() <= L - 1, (counts.max(), L)
    starts = np.zeros(P * NB, np.int64)
    starts[1:] = np.cumsum(counts)[:-1]
    pos = np.arange(len(r)) - starts[group]

    key = np.full((P, NB, L), -1.0, np.float32)
    val = np.zeros((P, NB, L), np.float32)
    idxk = np.full((P, NB, L), -1, np.int16)
    # keys globally distinct across i-blocks -> no false merges across block
    # boundaries within a partition row
    key[p, b, pos] = (b * OSH + r).astype(np.float32)
    val[p, b, pos] = v
    nonlast = np.zeros(len(r), bool)
    nonlast[:-1] = (group[:-1] == group[1:]) & (r[:-1] == r[1:])
    ent = r.astype(np.int16)
    ent[nonlast] = -1
    idxk[p, b, pos] = ent

    GW = GB * L
    sk = np.full((NGRP, P, GHOST + GW), -1.0, np.float32)
    sv = np.zeros((NGRP, P, GHOST + GW), np.float32)
    si = np.ascontiguousarray(idxk.reshape(P, NGRP, GW).transpose(1, 0, 2))
    sk[:, :, GHOST:] = key.reshape(P, NGRP, GW).transpose(1, 0, 2)
    sv[:, :, GHOST:] = val.reshape(P, NGRP, GW).transpose(1, 0, 2)
    return sk, sv, si


def _choose_L(rows, cols):
    worst = 0
    for q in range(8):
        m = (rows >= q * OSH) & (rows < (q + 1) * OSH)
        c = cols[m].astype(np.int64)
        group = (c % P) * np.int64(NB) + (c // P)
        counts = np.bincount(group, minlength=P * NB)
        worst = max(worst, int(counts.max()))
    L = worst + 2
    L += L % 2
    return L


def _prep_x(xf):
    a = xf.T.reshape(NB, P, NTOK)                       # [b, p, t]
    a = a.reshape(NB, P, NTS, TS).transpose(2, 1, 0, 3)  # [ts, p, b, u]
    return np.ascontiguousarray(a).astype(ml_dtypes.bfloat16)


def _make_in_maps(x, data, rows, cols, L):
    x = np.asarray(x, dtype=np.float32)
    data = np.asarray(data, dtype=np.float32)
    rows = np.asarray(rows).astype(np.int64)
    cols = np.asarray(cols).astype(np.int64)

    xf = np.ascontiguousarray(x.reshape(-1, IN_F))
    xs = _prep_x(xf)
    in_maps = []
    for core in range(8):
        sk, sv, si = _make_scatter_arrays(rows, cols, data, core * OSH, L)
        in_maps.append({"xs": xs, "sk": sk, "sv": sv, "si": si})
    return in_maps


def kernel(x, data, rows, cols):
    L = _choose_L(rows, cols)
    in_maps = _make_in_maps(x, data, rows, cols, L)
    nc = _build_nc(L)
    res = run_bass_kernel_spmd(nc, in_maps, core_ids=list(range(8)))

    y = np.zeros((NTOK, OUT_F), np.float32)
    for core in range(8):
        y[:, core * OSH:(core + 1) * OSH] = \
            res.results[core]["y"].astype(np.float32).T
    return y.reshape(BATCH, SEQ, OUT_F)


# revision 6
# speedup vs baseline: 346.9165x; 1.0341x over previous
"""Trainium2 Bass kernel for nn_CuPyLinear: y = x @ W^T with W given as a
sparse COO weight (data, rows, cols), W [4096, 4096] ~10% dense,
x [2, 2048, 4096] fp32, y [2, 2048, 4096] fp32.

Strategy (8 NeuronCores, SPMD single NEFF): core c -> output-feature octant
(512 rows of W); every core sees all 4096 tokens.  Host-side work is layout
only: shard/sort/pad the COO indices, transpose/cast x; every arithmetic
combination of data values (duplicate-COO summation, densification, matmul)
runs on device.

Per-core device program:
  - Scatter arrays in NGRP=4 groups of GB=8 i-blocks (an i-block is 128
    input features):
      sk/sv [NGRP, 128, GHOST+GB*L] fp32: sort key / COO value lists, where
        the entry for W[r, c] sits in partition p = c % 128, i-block
        b = c // 128, sorted by r; duplicate (r, c) entries are adjacent.
        Pads/ghost use key = -1, val = 0.
      si [NGRP, 128, GB*L] int16: scatter index (r offset within the
        octant); for duplicate runs all but the LAST entry are -1 (killed).
  - Duplicate merge per group on VectorE: mask = (key == shift1(key)), then
    tensor_tensor_scan computes segmented running sums; the last entry of
    each duplicate run holds the full sum.  Output downcast to bf16.
  - GpSimd local_scatter builds dense W^T tiles wt[b] [128 i x 512 o] bf16.
  - TensorE: yT[o, t] += wt[b][:, ob*128:+128].T @ x[b-slice, ts*512:+512]
    with full-K accumulation in PSUM (32 i-blocks per accumulation group).
    Token slices ts0/ts1 are computed DURING the scatter phase (their
    matmuls chase the scatter group by group); ts2..7 run b-inner after.
  - y written once per (ob, ts) tile: PSUM -> bf16 -> DMA.  No DRAM
    accumulation.
  - Host assembles y from the per-core yT slices (cast fp32, transpose).
"""

import numpy as np
import ml_dtypes

import concourse.bacc as bacc
import concourse.mybir as mybir
import concourse.tile as tile
from concourse import library_config
from concourse.bass_utils import run_bass_kernel_spmd

P = 128
IN_F = 4096
OUT_F = 4096
BATCH, SEQ = 2, 2048
NTOK = BATCH * SEQ
NB = 32            # i-blocks of 128 input features
GHOST = 16
OSH = 512          # out features per core (octant)
NGRP = 4
GB = NB // NGRP    # i-blocks per merge group
NOB = OSH // P     # output 128-blocks per core (4)
TS = 512           # token slice width
NTS = NTOK // TS   # token slices (8)
NTS_A = 2          # token slices computed during the scatter phase


def _build_nc(L, reps=1):
    GW = GB * L
    SW = GHOST + GW
    nc = bacc.Bacc("TRN2", target_bir_lowering=False, debug=False, num_devices=8)
    dt = mybir.dt
    xs_d = nc.dram_tensor("xs", [NTS, P, NB, TS], dt.bfloat16,
                          kind="ExternalInput").ap()
    sk_d = nc.dram_tensor("sk", [NGRP, P, SW], dt.float32,
                          kind="ExternalInput").ap()
    sv_d = nc.dram_tensor("sv", [NGRP, P, SW], dt.float32,
                          kind="ExternalInput").ap()
    si_d = nc.dram_tensor("si", [NGRP, P, GW], dt.int16,
                          kind="ExternalInput").ap()
    y_d = nc.dram_tensor("y", [OSH, NTOK], dt.bfloat16,
                         kind="ExternalOutput").ap()

    with tile.TileContext(nc) as tc:
        with tc.tile_pool(name="mrg", bufs=2) as mrgp, \
             tc.tile_pool(name="scat", bufs=2) as scatp, \
             tc.tile_pool(name="wt", bufs=1) as wtp, \
             tc.tile_pool(name="xstr", bufs=3) as xsp, \
             tc.tile_pool(name="st", bufs=3) as stp, \
             tc.tile_pool(name="psum", bufs=2, space="PSUM") as psp:

            nc.gpsimd.load_library(library_config.local_scatter)

            wt = [wtp.tile([P, OSH], dt.bfloat16, name=f"wt{b}")
                  for b in range(NB)]

            for _ in range(reps):
                # x for the overlap slices, on the scalar-engine DMA queue so
                # the sync queue serves the scatter arrays first.
                xst = {}
                for ts in range(NTS_A):
                    xst[ts] = xsp.tile([P, NB, TS], dt.bfloat16, name="xst")
                    nc.scalar.dma_start(xst[ts][:], xs_d[ts])

                pacc = [psp.tile([P, NOB, TS], dt.float32, name="pacc")
                        for _ in range(NTS_A)]

                # Phase 1 + A: scatter group g, then immediately chase it
                # with the (ts0, ts1) matmuls over that group's i-blocks.
                for g in range(NGRP):
                    skt = scatp.tile([P, SW], dt.float32, name="skt")
                    svt = scatp.tile([P, SW], dt.float32, name="svt")
                    sit = scatp.tile([P, GW], dt.int16, name="sit")
                    nc.sync.dma_start(skt[:], sk_d[g])
                    nc.sync.dma_start(svt[:], sv_d[g])
                    nc.sync.dma_start(sit[:], si_d[g])
                    tm = mrgp.tile([P, GW], dt.float32, name="mask")
                    nc.vector.tensor_tensor(
                        out=tm[:], in0=skt[:, GHOST:], in1=skt[:, GHOST - 1:SW - 1],
                        op=mybir.AluOpType.is_equal)
                    vb = mrgp.tile([P, GW], dt.bfloat16, name="vb")
                    nc.vector.tensor_tensor_scan(
                        out=vb[:], data0=tm[:], data1=svt[:, GHOST:],
                        initial=0.0, op0=mybir.AluOpType.mult,
                        op1=mybir.AluOpType.add)
                    for j in range(GB):
                        b = g * GB + j
                        nc.gpsimd.local_scatter(
                            wt[b][:], vb[:, j * L:(j + 1) * L],
                            sit[:, j * L:(j + 1) * L],
                            channels=P, num_elems=OSH, num_idxs=L)
                    # j descending: the first matmul waits on the group's
                    # last scatter tick, every later wait is dominated and
                    # elided by the tile scheduler.
                    for ts in range(NTS_A):
                        for ob in range(NOB):
                            for j in range(GB - 1, -1, -1):
                                b = g * GB + j
                                nc.tensor.matmul(
                                    pacc[ts][:, ob, :],
                                    lhsT=wt[b][:, ob * P:(ob + 1) * P],
                                    rhs=xst[ts][:, b, :],
                                    start=(g == 0 and j == GB - 1),
                                    stop=(g == NGRP - 1 and j == 0))

                for ts in range(NTS_A):
                    st = stp.tile([P, NOB, TS], dt.bfloat16, name="st")
                    nc.vector.tensor_copy(st[:], pacc[ts][:])
                    for ob in range(NOB):
                        nc.sync.dma_start(
                            y_d[ob * P:(ob + 1) * P, ts * TS:(ts + 1) * TS],
                            st[:, ob, :])

                # Phase B: remaining token slices, full-K b-inner.
                for ts in range(NTS_A, NTS):
                    xt = xsp.tile([P, NB, TS], dt.bfloat16, name="xst")
                    nc.scalar.dma_start(xt[:], xs_d[ts])
                    pt = psp.tile([P, NOB, TS], dt.float32, name="pacc")
                    for ob in range(NOB):
                        for b in range(NB - 1, -1, -1):
                            nc.tensor.matmul(
                                pt[:, ob, :],
                                lhsT=wt[b][:, ob * P:(ob + 1) * P],
                                rhs=xt[:, b, :],
                                start=(b == NB - 1), stop=(b == 0))
                    st = stp.tile([P, NOB, TS], dt.bfloat16, name="st")
                    nc.vector.tensor_copy(st[:], pt[:])
                    for ob in range(NOB):
                        nc.sync.dma_start(
                            y_d[ob * P:(ob + 1) * P, ts * TS:(ts + 1) * TS],
                            st[:, ob, :])

    nc.compile()
    return nc


def _make_scatter_arrays(rows, cols, data, o_base, L):
    m = (rows >= o_base) & (rows < o_base + OSH)
    r = (rows[m] - o_base).astype(np.int64)
    c = cols[m].astype(np.int64)
    v = data[m].astype(np.float32)
    p = c % P
    b = c // P
    group = p * NB + b
    order = np.lexsort((r, group))
    r, v, p, b, group = r[order], v[order], p[order], b[order], group[order]
    counts = np.bincount(group, minlength=P * NB)
    assert counts.max() <= L - 1, (counts.max(), L)
    starts = np.zeros(P * NB, np.int64)
    starts[1:] = np.cumsum(counts)[:-1]
    pos = np.arange(len(r)) - starts[group]

    key = np.full((P, NB, L), -1.0, np.float32)
    val = np.zeros((P, NB, L), np.float32)
    idxk = np.full((P, NB, L), -1, np.int16)
    # keys globally distinct across i-blocks -> no false merges across block
    # boundaries within a partition row
    key[p, b, pos] = (b * OSH + r).astype(np.float32)
    val[p, b, pos] = v
    nonlast = np.zeros(len(r), bool)
    nonlast[:-1] = (group[:-1] == group[1:]) & (r[:-1] == r[1:])
    ent = r.astype(np.int16)
    ent[nonlast] = -1
    idxk[p, b, pos] = ent

    GW = GB * L
    sk = np.full((NGRP, P, GHOST + GW), -1.0, np.float32)
    sv = np.zeros((NGRP, P, GHOST + GW), np.float32)
    si = np.ascontiguousarray(idxk.reshape(P, NGRP, GW).transpose(1, 0, 2))
    sk[:, :, GHOST:] = key.reshape(P, NGRP, GW).transpose(1, 0, 2)
    sv[:, :, GHOST:] = val.reshape(P, NGRP, GW).transpose(1, 0, 2)
    return sk, sv, si


def _choose_L(rows, cols):
    worst = 0
    for q in range(8):
        m = (rows >= q * OSH) & (rows < (q + 1) * OSH)
        c = cols[m].astype(np.int64)
        group = (c % P) * np.int64(NB) + (c // P)
        counts = np.bincount(group, minlength=P * NB)
        worst = max(worst, int(counts.max()))
    L = worst + 2
    L += L % 2
    return L


def _prep_x(xf):
    a = xf.T.reshape(NB, P, NTOK)                       # [b, p, t]
    a = a.reshape(NB, P, NTS, TS).transpose(2, 1, 0, 3)  # [ts, p, b, u]
    return np.ascontiguousarray(a).astype(ml_dtypes.bfloat16)


def _make_in_maps(x, data, rows, cols, L):
    x = np.asarray(x, dtype=np.float32)
    data = np.asarray(data, dtype=np.float32)
    rows = np.asarray(rows).astype(np.int64)
    cols = np.asarray(cols).astype(np.int64)

    xf = np.ascontiguousarray(x.reshape(-1, IN_F))
    xs = _prep_x(xf)
    in_maps = []
    for core in range(8):
        sk, sv, si = _make_scatter_arrays(rows, cols, data, core * OSH, L)
        in_maps.append({"xs": xs, "sk": sk, "sv": sv, "si": si})
    return in_maps


def kernel(x, data, rows, cols):
    L = _choose_L(rows, cols)
    in_maps = _make_in_maps(x, data, rows, cols, L)
    nc = _build_nc(L)
    res = run_bass_kernel_spmd(nc, in_maps, core_ids=list(range(8)))

    y = np.zeros((NTOK, OUT_F), np.float32)
    for core in range(8):
        y[:, core * OSH:(core + 1) * OSH] = \
            res.results[core]["y"].astype(np.float32).T
    return y.reshape(BATCH, SEQ, OUT_F)


# revision 10
# speedup vs baseline: 420.8697x; 1.2132x over previous
"""Trainium2 Bass kernel for nn_CuPyLinear: y = x @ W^T with W given as a
sparse COO weight (data, rows, cols), W [4096, 4096] ~10% dense,
x [2, 2048, 4096] fp32, y [2, 2048, 4096] fp32.

Strategy (8 NeuronCores, SPMD single NEFF): core c -> output-feature octant
(512 rows of W); every core sees all 4096 tokens.  Host-side work is layout
only: shard/sort/pad the COO indices, transpose/cast x; every arithmetic
combination of data values (duplicate-COO summation, densification, matmul)
runs on device.

Per-core device program:
  - Scatter arrays in NGRP=4 groups of GB=8 i-blocks (an i-block is 128
    input features):
      sk/sv [NGRP, 128, GHOST+GB*L] fp32: sort key / COO value lists, where
        the entry for W[r, c] sits in partition p = c % 128, i-block
        b = c // 128, sorted by r; duplicate (r, c) entries are adjacent.
        Pads/ghost use key = -1, val = 0.
      si [NGRP, 128, GB*L] int16: scatter index (r offset within the
        octant); for duplicate runs all but the LAST entry are -1 (killed).
  - Duplicate merge per group on VectorE: mask = (key == shift1(key)), then
    tensor_tensor_scan computes segmented running sums; the last entry of
    each duplicate run holds the full sum.  Output downcast to bf16.
  - GpSimd local_scatter builds dense W^T tiles wt[b] [128 i x 512 o] bf16.
  - TensorE: yT[o, t] += wt[b][:, ob*128:+128].T @ x[b-slice, ts*512:+512]
    with full-K accumulation in PSUM (32 i-blocks per accumulation group).
    Token slices ts0/ts1 are computed DURING the scatter phase (their
    matmuls chase the scatter group by group); ts2..7 run b-inner after.
  - y written once per (ob, ts) tile: PSUM -> bf16 -> DMA.  No DRAM
    accumulation.
  - Host assembles y from the per-core yT slices (cast fp32, transpose).
"""

import numpy as np
import ml_dtypes

import concourse.bacc as bacc
import concourse.mybir as mybir
import concourse.tile as tile
from concourse import library_config
from concourse.bass_utils import run_bass_kernel_spmd

P = 128
IN_F = 4096
OUT_F = 4096
BATCH, SEQ = 2, 2048
NTOK = BATCH * SEQ
NB = 32            # i-blocks of 128 input features
HB = NB // 2       # i-blocks per x half-tile
GHOST = 16
OSH = 512          # out features per core (octant)
NGRP = 4
GB = NB // NGRP    # i-blocks per merge group
NOB = OSH // P     # output 128-blocks per core (4)
TS = 512           # token slice width
NCH = 4            # token chunks (pairs of 512-slices)


def _build_nc(L, reps=1):
    GW = GB * L
    SW = GHOST + GW
    nc = bacc.Bacc("TRN2", target_bir_lowering=False, debug=False, num_devices=8)
    dt = mybir.dt
    xs_d = nc.dram_tensor("xs", [NCH, P, NB, 2, TS], dt.bfloat16,
                          kind="ExternalInput").ap()
    sk_d = nc.dram_tensor("sk", [NGRP, P, SW], dt.float32,
                          kind="ExternalInput").ap()
    sv_d = nc.dram_tensor("sv", [NGRP, P, SW], dt.float32,
                          kind="ExternalInput").ap()
    si_d = nc.dram_tensor("si", [NGRP, P, GW], dt.int16,
                          kind="ExternalInput").ap()
    y_d = nc.dram_tensor("y", [OSH, NTOK], dt.bfloat16,
                         kind="ExternalOutput").ap()

    with tile.TileContext(nc) as tc:
        with tc.tile_pool(name="mrg", bufs=2) as mrgp, \
             tc.tile_pool(name="scat", bufs=2) as scatp, \
             tc.tile_pool(name="wt", bufs=1) as wtp, \
             tc.tile_pool(name="xstr", bufs=3) as xsp, \
             tc.tile_pool(name="st", bufs=3) as stp, \
             tc.tile_pool(name="psum", bufs=4, space="PSUM") as psp:

            nc.gpsimd.load_library(library_config.local_scatter)

            wt = [wtp.tile([P, OSH], dt.bfloat16, name=f"wt{b}")
                  for b in range(NB)]

            def load_chunk(ch):
                # two half-tiles (i-blocks 16..31 first: matmuls run b
                # descending), on the scalar-engine DMA queue so the sync
                # queue serves the scatter arrays first.
                halves = []
                for hb in (1, 0):
                    xt = xsp.tile([P, HB, 2, TS], dt.bfloat16, name="xst")
                    nc.scalar.dma_start(xt[:], xs_d[ch, :, hb * HB:(hb + 1) * HB])
                    halves.append(xt)
                return halves[1], halves[0]   # [h0, h1]

            for _ in range(reps):
                xc0 = load_chunk(0)

                pacc = [psp.tile([P, 2, TS], dt.float32, name="pacc")
                        for _ in range(NOB)]

                # Phase 1 + A: scatter group g, then immediately chase it
                # with chunk-0 matmuls over that group's i-blocks.
                for g in range(NGRP):
                    skt = scatp.tile([P, SW], dt.float32, name="skt")
                    svt = scatp.tile([P, SW], dt.float32, name="svt")
                    sit = scatp.tile([P, GW], dt.int16, name="sit")
                    nc.sync.dma_start(skt[:], sk_d[g])
                    nc.sync.dma_start(svt[:], sv_d[g])
                    nc.sync.dma_start(sit[:], si_d[g])
                    tm = mrgp.tile([P, GW], dt.float32, name="mask")
                    nc.vector.tensor_tensor(
                        out=tm[:], in0=skt[:, GHOST:], in1=skt[:, GHOST - 1:SW - 1],
                        op=mybir.AluOpType.is_equal)
                    vb = mrgp.tile([P, GW], dt.bfloat16, name="vb")
                    nc.vector.tensor_tensor_scan(
                        out=vb[:], data0=tm[:], data1=svt[:, GHOST:],
                        initial=0.0, op0=mybir.AluOpType.mult,
                        op1=mybir.AluOpType.add)
                    for j in range(GB):
                        b = g * GB + j
                        nc.gpsimd.local_scatter(
                            wt[b][:], vb[:, j * L:(j + 1) * L],
                            sit[:, j * L:(j + 1) * L],
                            channels=P, num_elems=OSH, num_idxs=L)
                    # j descending: the first matmul waits on the group's
                    # last scatter tick, every later wait is dominated and
                    # elided by the tile scheduler.  Each weight tile feeds
                    # both token slices of the chunk back-to-back so
                    # LDWEIGHTS is amortized over 2 matmuls.
                    for ob in range(NOB):
                        for j in range(GB - 1, -1, -1):
                            b = g * GB + j
                            xh = xc0[b // HB]
                            w = wt[b][:, ob * P:(ob + 1) * P]
                            for e in range(2):
                                nc.tensor.matmul(
                                    pacc[ob][:, e, :], lhsT=w,
                                    rhs=xh[:, b % HB, e, :],
                                    start=(g == 0 and j == GB - 1),
                                    stop=(g == NGRP - 1 and j == 0))

                for ob in range(NOB):
                    st = stp.tile([P, 2, TS], dt.bfloat16, name="st")
                    nc.vector.tensor_copy(st[:], pacc[ob][:])
                    for e in range(2):
                        nc.sync.dma_start(
                            y_d[ob * P:(ob + 1) * P, e * TS:(e + 1) * TS],
                            st[:, e, :])

                # Phase B: remaining chunks, full-K b-descending, weight
                # shared across the chunk's two token slices.
                for ch in range(1, NCH):
                    xc = load_chunk(ch)
                    for ob in range(NOB):
                        pt = psp.tile([P, 2, TS], dt.float32, name="pacc")
                        for b in range(NB - 1, -1, -1):
                            xh = xc[b // HB]
                            w = wt[b][:, ob * P:(ob + 1) * P]
                            for e in range(2):
                                nc.tensor.matmul(
                                    pt[:, e, :], lhsT=w,
                                    rhs=xh[:, b % HB, e, :],
                                    start=(b == NB - 1), stop=(b == 0))
                        st = stp.tile([P, 2, TS], dt.bfloat16, name="st")
                        nc.vector.tensor_copy(st[:], pt[:])
                        for e in range(2):
                            t0 = (ch * 2 + e) * TS
                            nc.sync.dma_start(
                                y_d[ob * P:(ob + 1) * P, t0:t0 + TS],
                                st[:, e, :])

    nc.compile()
    return nc


def _make_scatter_arrays(rows, cols, data, o_base, L):
    m = (rows >= o_base) & (rows < o_base + OSH)
    r = (rows[m] - o_base).astype(np.int64)
    c = cols[m].astype(np.int64)
    v = data[m].astype(np.float32)
    p = c % P
    b = c // P
    group = p * NB + b
    order = np.lexsort((r, group))
    r, v, p, b, group = r[order], v[order], p[order], b[order], group[order]
    counts = np.bincount(group, minlength=P * NB)
    assert counts.max() <= L - 1, (counts.max(), L)
    starts = np.zeros(P * NB, np.int64)
    starts[1:] = np.cumsum(counts)[:-1]
    pos = np.arange(len(r)) - starts[group]

    key = np.full((P, NB, L), -1.0, np.float32)
    val = np.zeros((P, NB, L), np.float32)
    idxk = np.full((P, NB, L), -1, np.int16)
    # keys globally distinct across i-blocks -> no false merges across block
    # boundaries within a partition row
    key[p, b, pos] = (b * OSH + r).astype(np.float32)
    val[p, b, pos] = v
    nonlast = np.zeros(len(r), bool)
    nonlast[:-1] = (group[:-1] == group[1:]) & (r[:-1] == r[1:])
    ent = r.astype(np.int16)
    ent[nonlast] = -1
    idxk[p, b, pos] = ent

    GW = GB * L
    sk = np.full((NGRP, P, GHOST + GW), -1.0, np.float32)
    sv = np.zeros((NGRP, P, GHOST + GW), np.float32)
    si = np.ascontiguousarray(idxk.reshape(P, NGRP, GW).transpose(1, 0, 2))
    sk[:, :, GHOST:] = key.reshape(P, NGRP, GW).transpose(1, 0, 2)
    sv[:, :, GHOST:] = val.reshape(P, NGRP, GW).transpose(1, 0, 2)
    return sk, sv, si


def _choose_L(rows, cols):
    worst = 0
    for q in range(8):
        m = (rows >= q * OSH) & (rows < (q + 1) * OSH)
        c = cols[m].astype(np.int64)
        group = (c % P) * np.int64(NB) + (c // P)
        counts = np.bincount(group, minlength=P * NB)
        worst = max(worst, int(counts.max()))
    L = worst + 2
    L += L % 2
    return L


def _prep_x(xf):
    a = xf.T.reshape(NB, P, NTOK)                        # [b, p, t]
    a = a.reshape(NB, P, NCH, 2, TS).transpose(2, 1, 0, 3, 4)  # [ch,p,b,e,u]
    return np.ascontiguousarray(a).astype(ml_dtypes.bfloat16)


def _make_in_maps(x, data, rows, cols, L):
    x = np.asarray(x, dtype=np.float32)
    data = np.asarray(data, dtype=np.float32)
    rows = np.asarray(rows).astype(np.int64)
    cols = np.asarray(cols).astype(np.int64)

    xf = np.ascontiguousarray(x.reshape(-1, IN_F))
    xs = _prep_x(xf)
    in_maps = []
    for core in range(8):
        sk, sv, si = _make_scatter_arrays(rows, cols, data, core * OSH, L)
        in_maps.append({"xs": xs, "sk": sk, "sv": sv, "si": si})
    return in_maps


def kernel(x, data, rows, cols):
    L = _choose_L(rows, cols)
    in_maps = _make_in_maps(x, data, rows, cols, L)
    nc = _build_nc(L)
    res = run_bass_kernel_spmd(nc, in_maps, core_ids=list(range(8)))

    y = np.zeros((NTOK, OUT_F), np.float32)
    for core in range(8):
        y[:, core * OSH:(core + 1) * OSH] = \
            res.results[core]["y"].astype(np.float32).T
    return y.reshape(BATCH, SEQ, OUT_F)
